# revision 1
# baseline (speedup 1.0000x reference)
"""Trainium2 Bass kernel: ConvNeXt MLP + parallel MoE-LoRA (data-parallel over tokens).

Math per token t (D=512, Dh=2048, E=3 experts, r=8, top-k=2):
    base = gelu(x @ W1 + b1) @ W2 + b2
    g_e  = gelu(x @ w_down[e]) * wts[e, t]          (wts from top-k routing)
    out  = base + sum_e g_e @ w_up[e]

Strategy (per NeuronCore, 8 cores data-parallel on the token dim):
  - tokens tiled 128 at a time; supergroups of 4 tiles (512 tokens) so the
    MM1 moving free dim is 512.
  - x tile [128t, 512d] is cast to bf16 and PE-transposed into xT [128d, t].
  - MM1: hT[h,t] = W1_chunk.T @ xT   (feature-major hidden), fused
    bias+gelu on ScalarE into actT (bf16).
  - MM2: out[t,d] accumulates 16 h-chunks (lhsT = actT slices) + the
    MoE-LoRA rank-24 matmul in one PSUM accumulation group; b2 is added
    during the PSUM->SBUF drain from a replicated [128,512] bias tile.
  - LoRA: g[t,24] = gelu(xT.T @ wdown_all), scaled per-expert by routing
    weights (per-partition scalars), PE-transposed, matmul'd with
    wup_all[24,512] into the same PSUM accumulator.
  - routing weights wts[e,t] = sum_k probs[t,k]*(idx[t,k]==e) computed on
    device in a small DVE prologue over all tokens at once.
  - matmuls run in bf16 (full PE rate, fast weight load); accumulation is
    always fp32 in PSUM.
"""

import os
import numpy as np

P = 128
D = 512
DH = 2048
E = 3
R = 8
ER = E * R  # 24
NH = DH // P  # 16
NDC = D // P  # 4
N_CORES = 8
T_FULL = 64 * 28 * 28  # 50176
TC = T_FULL // N_CORES  # 6272
GROUP_TILES = 4

_CACHE = {}


def _build(tc_tokens, use_gelu=True):
    import concourse.bacc as bacc
    import concourse.tile as tile
    import concourse.mybir as mybir
    from contextlib import ExitStack

    f32 = mybir.dt.float32
    bf16 = mybir.dt.bfloat16
    i32 = mybir.dt.int32
    AF = mybir.ActivationFunctionType
    act_fn = AF.Gelu if use_gelu else AF.Relu
    OP = mybir.AluOpType

    nt = tc_tokens // P  # token tiles
    assert tc_tokens % P == 0

    nc = bacc.Bacc("TRN2", target_bir_lowering=False, debug=False,
                   num_devices=N_CORES)

    x = nc.dram_tensor("x", [tc_tokens, D], f32, kind="ExternalInput").ap()
    w1 = nc.dram_tensor("w1", [D, DH], f32, kind="ExternalInput").ap()
    w2 = nc.dram_tensor("w2", [DH, D], f32, kind="ExternalInput").ap()
    b1 = nc.dram_tensor("b1", [DH], f32, kind="ExternalInput").ap()
    b2 = nc.dram_tensor("b2", [D], f32, kind="ExternalInput").ap()
    wd = nc.dram_tensor("wd", [D, ER], f32, kind="ExternalInput").ap()
    wu = nc.dram_tensor("wu", [ER, D], f32, kind="ExternalInput").ap()
    tkp = nc.dram_tensor("tkp", [tc_tokens, 2], f32, kind="ExternalInput").ap()
    tki = nc.dram_tensor("tki", [tc_tokens, 4], i32, kind="ExternalInput").ap()
    ident_d = nc.dram_tensor("ident", [P, P], f32, kind="ExternalInput").ap()
    out = nc.dram_tensor("out", [tc_tokens, D], f32, kind="ExternalOutput").ap()

    with tile.TileContext(nc) as tc, ExitStack() as ctx:
        cons = ctx.enter_context(tc.tile_pool(name="cons", bufs=1))
        stg = ctx.enter_context(tc.tile_pool(name="stg", bufs=4))
        xin = ctx.enter_context(tc.tile_pool(name="xin", bufs=12))
        xbp = ctx.enter_context(tc.tile_pool(name="xbp", bufs=4))
        xtp = ctx.enter_context(tc.tile_pool(name="xtp", bufs=4))
        actp = ctx.enter_context(tc.tile_pool(name="actp", bufs=3))
        outp = ctx.enter_context(tc.tile_pool(name="outp", bufs=4))
        gp = ctx.enter_context(tc.tile_pool(name="gp", bufs=8))
        ps_xt = ctx.enter_context(tc.tile_pool(name="ps_xt", bufs=2, space="PSUM"))
        ps_h = ctx.enter_context(tc.tile_pool(name="ps_h", bufs=2, space="PSUM"))
        ps_o = ctx.enter_context(tc.tile_pool(name="ps_o", bufs=2, space="PSUM"))
        ps_g = ctx.enter_context(tc.tile_pool(name="ps_g", bufs=1, space="PSUM"))
        ps_gt = ctx.enter_context(tc.tile_pool(name="ps_gt", bufs=1, space="PSUM"))

        # ---- identity (needed by the very first transposes) ----
        ident_sb = cons.tile([P, P], f32)
        nc.sync.dma_start(ident_sb[:], ident_d)
        ident_b = cons.tile([P, P], bf16)
        nc.vector.tensor_copy(ident_b[:], ident_sb[:])

        # ---- prefetch x tiles of group 0 before the weight DMAs ----
        prefetched = {}
        for i in range(min(2 * GROUP_TILES, nt)):
            x_t = xin.tile([P, D], f32, tag="x_t", name=f"x_t{i}")
            nc.sync.dma_start(x_t[:], x[i * P:(i + 1) * P, :])
            prefetched[i] = x_t

        # ---- transpose path helper (cast to bf16, PE transpose, pack xT) ----
        def emit_xpose_group(t0g, ngg, x_list):
            Gg = ngg * P
            xTt = xtp.tile([P, NDC * Gg], bf16, tag="xT", name=f"xT{t0g}")
            for i in range(ngg):
                xb = xbp.tile([P, D], bf16, tag="xb", name=f"xb{t0g + i}")
                nc.vector.tensor_copy(xb[:], x_list[i][:])
                px = ps_xt.tile([P, 512], bf16, tag="ps_x", name=f"ps_x{t0g + i}")
                for c in range(NDC):
                    nc.tensor.matmul(px[:, c * P:(c + 1) * P],
                                     xb[:, c * P:(c + 1) * P],
                                     ident_b[:],
                                     is_transpose=True,
                                     start=(c == 0), stop=(c == NDC - 1))
                nc.vector.tensor_copy(
                    xTt.rearrange("p (c g) -> p c g", c=NDC)[:, :, i * P:(i + 1) * P],
                    px.rearrange("p (c g) -> p c g", c=NDC))
            return xTt

        # groups 0-1 transposed at max priority so DVE serves them before
        # the weight casts / routing prologue (PE warms up immediately)
        xT_pre = {}
        with tc.high_priority():
            for t0p in (0, GROUP_TILES):
                if t0p < nt:
                    ngp = min(GROUP_TILES, nt - t0p)
                    xs = [prefetched.pop(t0p + i) for i in range(ngp)]
                    xT_pre[t0p] = emit_xpose_group(t0p, ngp, xs)

        # ---- weights: staged fp32 DMA -> bf16 cast ----
        W1s = cons.tile([P, NDC * DH], bf16)
        for c in range(NDC):
            s = stg.tile([P, DH], f32, tag="stage", name=f"stg_w1_{c}")
            nc.sync.dma_start(s[:], w1[c * P:(c + 1) * P, :])
            nc.vector.tensor_copy(W1s[:, c * DH:(c + 1) * DH], s[:])
        W2s = cons.tile([P, NH * D], bf16)
        for h in range(NH):
            s = stg.tile([P, D], f32, tag="stage", name=f"stg_w2_{h}")
            nc.sync.dma_start(s[:], w2[h * P:(h + 1) * P, :])
            nc.gpsimd.tensor_copy(W2s[:, h * D:(h + 1) * D], s[:])
        wdr = cons.tile([P, NDC * ER], bf16)
        for c in range(NDC):
            s = stg.tile([P, ER], f32, tag="stage", name=f"stg_wd_{c}")
            nc.sync.dma_start(s[:], wd[c * P:(c + 1) * P, :])
            nc.gpsimd.tensor_copy(wdr[:, c * ER:(c + 1) * ER], s[:])
        wur = cons.tile([ER, D], bf16)
        s = stg.tile([ER, D], f32, tag="stage", name="stg_wu")
        nc.sync.dma_start(s[:], wu)
        nc.gpsimd.tensor_copy(wur[:], s[:])
        b1_sb = cons.tile([P, NH], f32)
        nc.sync.dma_start(b1_sb[:], b1.rearrange("(c p) -> p c", p=P))

        # ---- replicated b2 [128, 512] via rank-1 ones x b2 matmul ----
        ones_b = cons.tile([1, P], bf16)
        nc.vector.memset(ones_b[:], 1.0)
        b2s = stg.tile([1, D], f32, tag="stage", name="stg_b2")
        nc.sync.dma_start(b2s[:], b2.rearrange("(o d) -> o d", o=1))
        b2b = cons.tile([1, D], bf16)
        nc.vector.tensor_copy(b2b[:], b2s[:])
        ps_b2 = ps_o.tile([P, D], f32, tag="ps_out", name="ps_b2rep")
        nc.tensor.matmul(ps_b2[:], ones_b[:], b2b[:], start=True, stop=True)
        b2rep = cons.tile([P, D], f32)
        nc.vector.tensor_copy(b2rep[:], ps_b2[:])

        # ---- routing weights prologue: wts[e, tile, p] ----
        tp_sb = cons.tile([P, nt * 2], f32)
        nc.sync.dma_start(tp_sb.rearrange("p (n k) -> p n k", k=2),
                          tkp.rearrange("(n p) k -> p n k", p=P))
        ti_sb = cons.tile([P, nt * 4], i32)
        nc.sync.dma_start(ti_sb.rearrange("p (n k) -> p n k", k=4),
                          tki.rearrange("(n p) k -> p n k", p=P))
        idxf = cons.tile([P, nt * 2], f32)
        iv = ti_sb.rearrange("p (n k two) -> p n k two", k=2, two=2)
        nc.vector.tensor_copy(
            idxf.rearrange("p (n k one) -> p n k one", k=2, one=1),
            iv[:, :, :, 0:1])
        wts = cons.tile([P, E * nt], f32)
        for e in range(E):
            eq = cons.tile([P, nt * 2], f32, tag="eq", name=f"eq{e}")
            nc.vector.tensor_scalar(eq[:], idxf[:], float(e), None,
                                    op0=OP.is_equal)
            nc.vector.tensor_tensor(eq[:], eq[:], tp_sb[:], op=OP.mult)
            nc.vector.tensor_reduce(wts[:, e * nt:(e + 1) * nt],
                                    eq.rearrange("p (n k) -> p n k", k=2),
                                    axis=mybir.AxisListType.X, op=OP.add)

        # ---- main loop over supergroups ----
        t0 = 0
        while t0 < nt:
            ng = min(GROUP_TILES, nt - t0)
            G = ng * P

            # prefetch the group after next so DMA stays ahead of compute
            for pf in range(t0 + 2 * GROUP_TILES, min(t0 + 3 * GROUP_TILES, nt)):
                if pf not in prefetched:
                    x_t = xin.tile([P, D], f32, tag="x_t", name=f"x_t{pf}")
                    nc.sync.dma_start(x_t[:], x[pf * P:(pf + 1) * P, :])
                    prefetched[pf] = x_t
            if t0 in xT_pre:
                xT = xT_pre.pop(t0)
            else:
                x_ts = []
                for i in range(ng):
                    tt = t0 + i
                    if tt in prefetched:
                        x_t = prefetched.pop(tt)
                    else:
                        x_t = xin.tile([P, D], f32, tag="x_t", name=f"x_t{tt}")
                        nc.sync.dma_start(x_t[:], x[tt * P:(tt + 1) * P, :])
                    x_ts.append(x_t)
                xT = emit_xpose_group(t0, ng, x_ts)


            # MM1 + bias + gelu -> actT [128h, (h_chunk, t)] bf16
            actT = actp.tile([P, NH * G], bf16, tag="actT", name=f"actT{t0}")
            for h in range(NH):
                ph = ps_h.tile([P, 512], f32, tag="ps_hT", name=f"ps_hT{t0}_{h}")
                for c in range(NDC):
                    nc.tensor.matmul(
                        ph[:, :G],
                        W1s[:, c * DH + h * P: c * DH + (h + 1) * P],
                        xT[:, c * G:(c + 1) * G],
                        start=(c == 0), stop=(c == NDC - 1))
                nc.scalar.activation(actT[:, h * G:(h + 1) * G], ph[:, :G],
                                     act_fn, bias=b1_sb[:, h:h + 1], scale=1.0)

            # LoRA chain for all tiles of the group (hides under MM1/MM2)
            gts = []
            for i in range(ng):
                tt = t0 + i
                pg = ps_g.tile([P, ER], f32, tag="ps_lg", name=f"ps_lg{tt}")
                for c in range(NDC):
                    nc.tensor.matmul(
                        pg[:],
                        xT[:, c * G + i * P: c * G + (i + 1) * P],
                        wdr[:, c * ER:(c + 1) * ER],
                        start=(c == 0), stop=(c == NDC - 1))
                g_sb = gp.tile([P, ER], f32, tag="g_sb", name=f"g_sb{tt}")
                nc.scalar.activation(g_sb[:], pg[:], act_fn)
                g2 = gp.tile([P, ER], bf16, tag="g2", name=f"g2{tt}")
                for e in range(E):
                    nc.vector.tensor_scalar(
                        g2[:, e * R:(e + 1) * R], g_sb[:, e * R:(e + 1) * R],
                        wts[:, e * nt + tt: e * nt + tt + 1], None, op0=OP.mult)
                pgt = ps_gt.tile([ER, P], bf16, tag="ps_lgt", name=f"ps_lgt{tt}")
                nc.tensor.matmul(pgt[:], g2[:], ident_b[:], is_transpose=True)
                gt_sb = gp.tile([ER, P], bf16, tag="gt_sb", name=f"gt_sb{tt}")
                nc.vector.tensor_copy(gt_sb[:], pgt[:])
                gts.append(gt_sb)

            # MM2 per token tile
            for i in range(ng):
                tt = t0 + i
                po = ps_o.tile([P, D], f32, tag="ps_out", name=f"ps_out{tt}")
                for h in range(NH):
                    nc.tensor.matmul(
                        po[:],
                        actT[:, h * G + i * P: h * G + (i + 1) * P],
                        W2s[:, h * D:(h + 1) * D],
                        start=(h == 0), stop=False)
                # LoRA up into the same accumulator (closes the group)
                nc.tensor.matmul(po[:], gts[i][:], wur[:],
                                 start=False, stop=True)

                o_sb = outp.tile([P, D], f32, tag="o_sb", name=f"o_sb{tt}")
                nc.vector.tensor_tensor(o_sb[:], po[:], b2rep[:], op=OP.add)
                nc.sync.dma_start(out[tt * P:(tt + 1) * P, :], o_sb[:])

            t0 += ng

    nc.compile()
    return nc


def _get_nc():
    key = ("full", TC)
    if key not in _CACHE:
        _CACHE[key] = _build(TC, use_gelu=True)
    return _CACHE[key]


def _make_in_maps(inputs, tc_tokens=TC, n_cores=N_CORES):
    x = np.ascontiguousarray(inputs["x"], dtype=np.float32)
    T = x.size // D
    x_flat = x.reshape(T, D)
    W1 = np.ascontiguousarray(inputs["W1"], dtype=np.float32)
    W2 = np.ascontiguousarray(inputs["W2"], dtype=np.float32)
    b1 = np.ascontiguousarray(inputs["b1"], dtype=np.float32)
    b2 = np.ascontiguousarray(inputs["b2"], dtype=np.float32)
    wdn = np.ascontiguousarray(
        np.asarray(inputs["w_down"], dtype=np.float32).transpose(1, 0, 2).reshape(D, ER))
    wup = np.ascontiguousarray(
        np.asarray(inputs["w_up"], dtype=np.float32).reshape(ER, D))
    tkp = np.ascontiguousarray(inputs["topk_probs"], dtype=np.float32)
    tki_in = np.asarray(inputs["topk_indices"])
    tki = np.zeros((T, 4), dtype=np.int32)
    tki[:, 0] = tki_in[:, 0]
    tki[:, 2] = tki_in[:, 1]
    ident = np.eye(P, dtype=np.float32)

    in_maps = []
    for c in range(n_cores):
        sl = slice(c * tc_tokens, (c + 1) * tc_tokens)
        in_maps.append(dict(
            x=np.ascontiguousarray(x_flat[sl]), w1=W1, w2=W2, b1=b1, b2=b2,
            wd=wdn, wu=wup, tkp=np.ascontiguousarray(tkp[sl]),
            tki=np.ascontiguousarray(tki[sl]), ident=ident))
    return in_maps


def _ensure_ntff_hook():
    """Register the axon NTFF profile hook if the image's antenv lacks it."""
    import sys
    import types
    try:
        from antenv.axon_hooks import get_axon_ntff_profile_hook  # noqa: F401
        return True
    except ImportError:
        pass
    try:
        from trn_agent_boot.trn_boot import _ntff_profile_via_ctypes
        mod = types.ModuleType("antenv.axon_hooks")
        _hook = [None]
        mod.set_axon_ntff_profile_hook = lambda h: _hook.__setitem__(0, h)
        mod.get_axon_ntff_profile_hook = lambda: _hook[0]
        sys.modules["antenv.axon_hooks"] = mod
        import antenv
        antenv.axon_hooks = mod
        mod.set_axon_ntff_profile_hook(
            _ntff_profile_via_ctypes("/opt/axon/libaxon_pjrt.so"))
        return True
    except Exception:
        return False


def kernel(**inputs):
    from concourse.bass_utils import run_bass_kernel_spmd

    nc = _get_nc()
    in_maps = _make_in_maps(inputs)
    trace = bool(int(os.environ.get("KERNEL_TRACE", "0")))
    if trace and not _ensure_ntff_hook():
        trace = False
    res = run_bass_kernel_spmd(nc, in_maps, list(range(N_CORES)), trace=trace)
    if trace:
        _CACHE["last_result"] = res
    out = np.concatenate([res.results[i]["out"] for i in range(N_CORES)], axis=0)
    return out.reshape(np.asarray(inputs["x"]).shape).astype(np.float32)



# revision 2
# speedup vs baseline: 1.1389x; 1.1389x over previous
"""Trainium2 Bass kernel: ConvNeXt MLP + parallel MoE-LoRA (data-parallel over tokens).

Math per token t (D=512, Dh=2048, E=3 experts, r=8, top-k=2):
    base = gelu(x @ W1 + b1) @ W2 + b2
    g_e  = gelu(x @ w_down[e]) * wts[e, t]          (wts from top-k routing)
    out  = base + sum_e g_e @ w_up[e]

Strategy (per NeuronCore, 8 cores data-parallel on the token dim):
  - all operands are pre-packed on the host into their SBUF layouts in bf16:
    x arrives pre-transposed as xT [D, T_core] so the PE never runs x
    transposes and the DVE never runs casts; weights arrive in matmul-ready
    chunk layouts; b2 arrives pre-replicated [128, 512].
  - tokens tiled 128 at a time; supergroups of 4 tiles (512 tokens) so the
    MM1 moving free dim is 512.
  - MM1: hT[h,t] = W1_chunk.T @ xT (feature-major hidden, 4 PSUM banks so
    the Gelu drain never stalls the accumulation cadence), fused bias+gelu
    on ScalarE into actT (bf16).
  - MM2: out[t,d] accumulates 16 h-chunks (lhsT = actT slices) + the
    MoE-LoRA rank-24 matmul in one PSUM accumulation group; b2 added during
    the PSUM->SBUF drain from the preloaded replicated bias tile.
  - LoRA for supergroup g+1 is computed during MM2 of supergroup g (down
    proj -> gelu -> routing scale -> PE transpose), so the close matmul of
    MM2 never waits on the scalar/vector chain.
  - routing weights wts[e,t] = sum_k probs[t,k]*(idx[t,k]==e) computed on
    device in a small DVE prologue over all tokens at once.
  - W1 is DMA'd in h-quarter pieces so MM1 starts after ~1MB of weight
    traffic; the Gelu activation table is pre-warmed by a dummy activation.
"""

import os
import numpy as np

P = 128
D = 512
DH = 2048
E = 3
R = 8
ER = E * R  # 24
NH = DH // P  # 16
NDC = D // P  # 4 d-chunks
N_CORES = 8
T_FULL = 64 * 28 * 28  # 50176
TC = T_FULL // N_CORES  # 6272
NT = TC // P  # 49 token tiles
GT = 4  # tiles per supergroup

_CACHE = {}


def _build():
    import concourse.bacc as bacc
    import concourse.tile as tile
    import concourse.mybir as mybir
    from contextlib import ExitStack

    f32 = mybir.dt.float32
    bf16 = mybir.dt.bfloat16
    i32 = mybir.dt.int32
    AF = mybir.ActivationFunctionType
    OP = mybir.AluOpType

    nt = NT
    groups = []
    t0 = 0
    while t0 < nt:
        ng = min(GT, nt - t0)
        groups.append((t0, ng))
        t0 += ng
    ngrp = len(groups)

    nc = bacc.Bacc("TRN2", target_bir_lowering=False, debug=False,
                   num_devices=N_CORES)

    xt = nc.dram_tensor("xt", [D, TC], bf16, kind="ExternalInput").ap()
    w1 = nc.dram_tensor("w1", [P, 4 * DH], bf16, kind="ExternalInput").ap()
    w2 = nc.dram_tensor("w2", [P, NH * D], bf16, kind="ExternalInput").ap()
    wd = nc.dram_tensor("wd", [P, NDC * ER], bf16, kind="ExternalInput").ap()
    wu = nc.dram_tensor("wu", [ER, D], bf16, kind="ExternalInput").ap()
    b1 = nc.dram_tensor("b1", [P, NH], f32, kind="ExternalInput").ap()
    b2 = nc.dram_tensor("b2", [P, D], f32, kind="ExternalInput").ap()
    tkp = nc.dram_tensor("tkp", [TC, 2], f32, kind="ExternalInput").ap()
    tki = nc.dram_tensor("tki", [TC, 4], i32, kind="ExternalInput").ap()
    ident_d = nc.dram_tensor("ident", [P, P], bf16, kind="ExternalInput").ap()
    out = nc.dram_tensor("out", [TC, D], f32, kind="ExternalOutput").ap()

    with tile.TileContext(nc) as tc, ExitStack() as ctx:
        cons = ctx.enter_context(tc.tile_pool(name="cons", bufs=1))
        xtp = ctx.enter_context(tc.tile_pool(name="xtp", bufs=4))
        actp = ctx.enter_context(tc.tile_pool(name="actp", bufs=2))
        outp = ctx.enter_context(tc.tile_pool(name="outp", bufs=4))
        lp = ctx.enter_context(tc.tile_pool(name="lp", bufs=4))
        gtp = ctx.enter_context(tc.tile_pool(name="gtp", bufs=8))
        ps_h = ctx.enter_context(tc.tile_pool(name="ps_h", bufs=4, space="PSUM"))
        ps_o = ctx.enter_context(tc.tile_pool(name="ps_o", bufs=2, space="PSUM"))
        ps_g = ctx.enter_context(tc.tile_pool(name="ps_g", bufs=1, space="PSUM"))
        ps_t = ctx.enter_context(tc.tile_pool(name="ps_t", bufs=1, space="PSUM"))

        # preload the Gelu activation table before the first real activation
        warm_in = cons.tile([1, 8], f32)
        nc.vector.memset(warm_in[:], 0.125)
        warm_out = cons.tile([1, 8], f32)
        nc.scalar.activation(warm_out[:], warm_in[:], AF.Gelu)

        # ---- DMAs in issue order (earlier == higher effective priority) ----
        ident_sb = cons.tile([P, P], bf16)
        nc.sync.dma_start(ident_sb[:], ident_d)
        tp_sb = cons.tile([P, nt * 2], f32)
        nc.sync.dma_start(tp_sb.rearrange("p (n k) -> p n k", k=2),
                          tkp.rearrange("(n p) k -> p n k", p=P))
        ti_sb = cons.tile([P, nt * 4], i32)
        nc.sync.dma_start(ti_sb.rearrange("p (n k) -> p n k", k=4),
                          tki.rearrange("(n p) k -> p n k", p=P))

        xt_r = xt.rearrange("(c p) t -> p c t", p=P)
        xT_tiles = {}

        def dma_xt(g):
            t0g, ngg = groups[g]
            G = ngg * P
            xTt = xtp.tile([P, NDC * G], bf16, tag="xT", name=f"xT{g}")
            nc.sync.dma_start(xTt.rearrange("p (c t) -> p c t", c=NDC),
                              xt_r[:, :, t0g * P:t0g * P + G])
            xT_tiles[g] = xTt

        dma_xt(0)
        W1s = cons.tile([P, 4 * DH], bf16)
        nc.sync.dma_start(W1s[:, 0:DH], w1[:, 0:DH])  # h-quarter 0
        dma_xt(1)
        for q in (1, 2, 3):
            nc.sync.dma_start(W1s[:, q * DH:(q + 1) * DH],
                              w1[:, q * DH:(q + 1) * DH])
        wdr = cons.tile([P, NDC * ER], bf16)
        nc.sync.dma_start(wdr[:], wd)
        wur = cons.tile([ER, D], bf16)
        nc.sync.dma_start(wur[:], wu)
        b1s = cons.tile([P, NH], f32)
        nc.sync.dma_start(b1s[:], b1)
        b2rep = cons.tile([P, D], f32)
        nc.sync.dma_start(b2rep[:], b2)
        W2s = cons.tile([P, NH * D], bf16)
        for j in range(4):
            nc.sync.dma_start(W2s[:, j * 4 * D:(j + 1) * 4 * D],
                              w2[:, j * 4 * D:(j + 1) * 4 * D])

        # ---- routing weights wts[p, e*nt + tile] (DVE) ----
        idxf = cons.tile([P, nt * 2], f32)
        iv = ti_sb.rearrange("p (n k two) -> p n k two", k=2, two=2)
        nc.vector.tensor_copy(
            idxf.rearrange("p (n k one) -> p n k one", k=2, one=1),
            iv[:, :, :, 0:1])
        wts = cons.tile([P, E * nt], f32)
        for e in range(E):
            eq = cons.tile([P, nt * 2], f32, tag="eq", name=f"eq{e}", bufs=2)
            nc.vector.tensor_scalar(eq[:], idxf[:], float(e), None,
                                    op0=OP.is_equal)
            nc.vector.tensor_tensor(eq[:], eq[:], tp_sb[:], op=OP.mult)
            nc.vector.tensor_reduce(wts[:, e * nt:(e + 1) * nt],
                                    eq.rearrange("p (n k) -> p n k", k=2),
                                    axis=mybir.AxisListType.X, op=OP.add)

        # ---- emit helpers ----
        gts = {}  # group -> list of [ER, P] bf16 lhsT tiles for the MM2 close

        def emit_mm1_chunk(g, h, actT):
            t0g, ngg = groups[g]
            G = ngg * P
            q, hh = h // 4, h % 4
            xTt = xT_tiles[g]
            ph = ps_h.tile([P, 512], f32, tag="ph", name=f"ph{g}_{h}")
            base = q * DH + hh * P
            for c in range(NDC):
                nc.tensor.matmul(
                    ph[:, :G],
                    W1s[:, base + c * 512:base + c * 512 + P],
                    xTt[:, c * G:(c + 1) * G],
                    start=(c == 0), stop=(c == NDC - 1))
            nc.scalar.activation(actT[:, h * G:(h + 1) * G], ph[:, :G],
                                 AF.Gelu, bias=b1s[:, h:h + 1], scale=1.0)

        def emit_lora_down(g):
            t0g, ngg = groups[g]
            G = ngg * P
            xTt = xT_tiles[g]
            pga = ps_g.tile([P, GT * ER], f32, tag="pg", name=f"pg{g}")
            for i in range(ngg):
                for c in range(NDC):
                    nc.tensor.matmul(
                        pga[:, i * ER:(i + 1) * ER],
                        xTt[:, c * G + i * P:c * G + (i + 1) * P],
                        wdr[:, c * ER:(c + 1) * ER],
                        start=(c == 0), stop=(c == NDC - 1))
            g2s = []
            for i in range(ngg):
                tt = t0g + i
                g_sb = lp.tile([P, ER], f32, tag="g_sb", name=f"g_sb{tt}")
                nc.scalar.activation(g_sb[:], pga[:, i * ER:(i + 1) * ER],
                                     AF.Gelu)
                g2 = lp.tile([P, ER], bf16, tag="g2", name=f"g2{tt}")
                for e in range(E):
                    nc.vector.tensor_scalar(
                        g2[:, e * R:(e + 1) * R], g_sb[:, e * R:(e + 1) * R],
                        wts[:, e * nt + tt:e * nt + tt + 1], None,
                        op0=OP.mult)
                g2s.append(g2)
            return g2s

        def emit_lora_t(g, g2s):
            t0g, ngg = groups[g]
            pta = ps_t.tile([ER, GT * P], bf16, tag="pt", name=f"pt{g}")
            for i in range(ngg):
                nc.tensor.matmul(pta[:, i * P:(i + 1) * P], g2s[i][:],
                                 ident_sb[:], is_transpose=True)
            lst = []
            for i in range(ngg):
                gt = gtp.tile([ER, P], bf16, tag="gt", name=f"gt{t0g + i}")
                nc.vector.tensor_copy(gt[:], pta[:, i * P:(i + 1) * P])
                lst.append(gt)
            gts[g] = lst

        def emit_mm2_tile(g, i, actT):
            t0g, ngg = groups[g]
            G = ngg * P
            tt = t0g + i
            po = ps_o.tile([P, D], f32, tag="po", name=f"po{tt}")
            for h in range(NH):
                nc.tensor.matmul(
                    po[:],
                    actT[:, h * G + i * P:h * G + (i + 1) * P],
                    W2s[:, h * D:(h + 1) * D],
                    start=(h == 0), stop=False)
            nc.tensor.matmul(po[:], gts[g][i][:], wur[:],
                             start=False, stop=True)
            o_sb = outp.tile([P, D], f32, tag="o_sb", name=f"o_sb{tt}")
            nc.vector.tensor_tensor(o_sb[:], po[:], b2rep[:], op=OP.add)
            nc.sync.dma_start(out[tt * P:(tt + 1) * P, :], o_sb[:])

        # ---- main loop: LoRA for group g+1 runs under MM2 of group g ----
        g0_g2s = None
        for g in range(ngrp):
            t0g, ngg = groups[g]
            actT = actp.tile([P, NH * ngg * P], bf16, tag="actT",
                             name=f"actT{g}")
            for h in range(NH):
                emit_mm1_chunk(g, h, actT)
                # group 0's LoRA is folded into its own MM1 (routing weights
                # and wd land mid-MM1; later groups pipeline one group ahead)
                if g == 0 and h == 4:
                    g0_g2s = emit_lora_down(0)
                if g == 0 and h == 10:
                    emit_lora_t(0, g0_g2s)
            nxt_g2s = emit_lora_down(g + 1) if g + 1 < ngrp else None
            emit_mm2_tile(g, 0, actT)
            if nxt_g2s is not None:
                emit_lora_t(g + 1, nxt_g2s)
            for i in range(1, ngg):
                emit_mm2_tile(g, i, actT)
            if g + 2 < ngrp:
                dma_xt(g + 2)

    nc.compile()
    return nc


def _get_nc():
    if "nc" not in _CACHE:
        _CACHE["nc"] = _build()
    return _CACHE["nc"]


def _make_in_maps(inputs):
    import ml_dtypes
    bf16 = ml_dtypes.bfloat16

    x = np.asarray(inputs["x"], dtype=np.float32)
    T = x.size // D
    x_flat = x.reshape(T, D)
    W1 = np.asarray(inputs["W1"], dtype=np.float32)
    W2 = np.asarray(inputs["W2"], dtype=np.float32)
    b1 = np.asarray(inputs["b1"], dtype=np.float32)
    b2 = np.asarray(inputs["b2"], dtype=np.float32)
    wdn = np.asarray(inputs["w_down"], dtype=np.float32)
    wup = np.asarray(inputs["w_up"], dtype=np.float32)
    tkp = np.ascontiguousarray(inputs["topk_probs"], dtype=np.float32)
    tki_in = np.asarray(inputs["topk_indices"])

    # SBUF layouts, bf16 (shared by all cores)
    # W1 [D, DH] -> [p, q, c, hh, x]: lhsT slice (c,h=q*4+hh) is contiguous
    w1p = np.ascontiguousarray(
        W1.reshape(NDC, P, 4, 4, P).transpose(1, 2, 0, 3, 4).reshape(P, 4 * DH)
    ).astype(bf16)
    # W2 [DH, D] -> [p, n, d]
    w2p = np.ascontiguousarray(
        W2.reshape(NH, P, D).transpose(1, 0, 2).reshape(P, NH * D)).astype(bf16)
    # w_down [E, D, R] -> [D, E*R] -> [p, c, er]
    wdf = wdn.transpose(1, 0, 2).reshape(D, ER)
    wdp = np.ascontiguousarray(
        wdf.reshape(NDC, P, ER).transpose(1, 0, 2).reshape(P, NDC * ER)
    ).astype(bf16)
    wup_b = np.ascontiguousarray(wup.reshape(ER, D)).astype(bf16)
    b1p = np.ascontiguousarray(b1.reshape(NH, P).T)
    b2p = np.ascontiguousarray(np.broadcast_to(b2[None, :], (P, D)),
                               dtype=np.float32)
    ident = np.eye(P, dtype=np.float32).astype(bf16)

    in_maps = []
    for c in range(N_CORES):
        sl = slice(c * TC, (c + 1) * TC)
        xt_c = np.ascontiguousarray(x_flat[sl].T.astype(bf16))
        tki = np.zeros((TC, 4), dtype=np.int32)
        tki[:, 0] = tki_in[sl, 0]
        tki[:, 2] = tki_in[sl, 1]
        in_maps.append(dict(
            xt=xt_c, w1=w1p, w2=w2p, wd=wdp, wu=wup_b, b1=b1p, b2=b2p,
            tkp=np.ascontiguousarray(tkp[sl]), tki=tki, ident=ident))
    return in_maps


def _ensure_ntff_hook():
    """Register the axon NTFF profile hook if the image's antenv lacks it."""
    import sys
    import types
    try:
        from antenv.axon_hooks import get_axon_ntff_profile_hook  # noqa: F401
        return True
    except ImportError:
        pass
    try:
        from trn_agent_boot.trn_boot import _ntff_profile_via_ctypes
        mod = types.ModuleType("antenv.axon_hooks")
        _hook = [None]
        mod.set_axon_ntff_profile_hook = lambda h: _hook.__setitem__(0, h)
        mod.get_axon_ntff_profile_hook = lambda: _hook[0]
        sys.modules["antenv.axon_hooks"] = mod
        import antenv
        antenv.axon_hooks = mod
        mod.set_axon_ntff_profile_hook(
            _ntff_profile_via_ctypes("/opt/axon/libaxon_pjrt.so"))
        return True
    except Exception:
        return False


def kernel(**inputs):
    from concourse.bass_utils import run_bass_kernel_spmd

    nc = _get_nc()
    in_maps = _make_in_maps(inputs)
    trace = bool(int(os.environ.get("KERNEL_TRACE", "0")))
    if trace and not _ensure_ntff_hook():
        trace = False
    res = run_bass_kernel_spmd(nc, in_maps, list(range(N_CORES)), trace=trace)
    if trace:
        _CACHE["last_result"] = res
    out = np.concatenate([res.results[i]["out"] for i in range(N_CORES)], axis=0)
    return out.reshape(np.asarray(inputs["x"]).shape).astype(np.float32)


# revision 7
# speedup vs baseline: 1.1679x; 1.0254x over previous
"""Trainium2 Bass kernel: ConvNeXt MLP + parallel MoE-LoRA (data-parallel over tokens).

Math per token t (D=512, Dh=2048, E=3 experts, r=8, top-k=2):
    base = gelu(x @ W1 + b1) @ W2 + b2
    g_e  = gelu(x @ w_down[e]) * wts[e, t]          (wts from top-k routing)
    out  = base + sum_e g_e @ w_up[e]

Strategy (per NeuronCore, 8 cores data-parallel on the token dim):
  - all operands are pre-packed on the host into their SBUF layouts in bf16:
    x arrives pre-transposed as xT [D, T_core] so the PE never runs x
    transposes and the DVE never runs casts; weights arrive in matmul-ready
    chunk layouts; b2 arrives pre-replicated [128, 512].
  - tokens tiled 128 at a time; supergroups of 4 tiles (512 tokens) so the
    MM1 moving free dim is 512.
  - MM1: hT[h,t] = W1_chunk.T @ xT (feature-major hidden, 4 PSUM banks so
    the Gelu drain never stalls the accumulation cadence), fused bias+gelu
    on ScalarE into actT (bf16).
  - MM2: out[t,d] accumulates 16 h-chunks (lhsT = actT slices) + the
    MoE-LoRA rank-24 matmul in one PSUM accumulation group; b2 added during
    the PSUM->SBUF drain from the preloaded replicated bias tile.
  - LoRA for supergroup g+1 is computed during MM2 of supergroup g (down
    proj -> gelu -> routing scale -> PE transpose), so the close matmul of
    MM2 never waits on the scalar/vector chain.
  - routing weights wts[e,t] = sum_k probs[t,k]*(idx[t,k]==e) computed on
    device in a small DVE prologue over all tokens at once.
  - W1 is DMA'd in h-quarter pieces so MM1 starts after ~1MB of weight
    traffic; the Gelu activation table is pre-warmed by a dummy activation.
"""

import os
import numpy as np

P = 128
D = 512
DH = 2048
E = 3
R = 8
ER = E * R  # 24
NH = DH // P  # 16
NDC = D // P  # 4 d-chunks
N_CORES = 8
T_FULL = 64 * 28 * 28  # 50176
TC = T_FULL // N_CORES  # 6272
NT = TC // P  # 49 token tiles
GT = 4  # tiles per supergroup

_CACHE = {}


def _build():
    import concourse.bacc as bacc
    import concourse.tile as tile
    import concourse.mybir as mybir
    from contextlib import ExitStack

    f32 = mybir.dt.float32
    bf16 = mybir.dt.bfloat16
    i32 = mybir.dt.int32
    AF = mybir.ActivationFunctionType
    OP = mybir.AluOpType

    nt = NT
    groups = []
    t0 = 0
    while t0 < nt:
        ng = min(GT, nt - t0)
        groups.append((t0, ng))
        t0 += ng
    ngrp = len(groups)

    nc = bacc.Bacc("TRN2", target_bir_lowering=False, debug=False,
                   num_devices=N_CORES)

    xt = nc.dram_tensor("xt", [P, NDC * TC], bf16, kind="ExternalInput").ap()
    w1 = nc.dram_tensor("w1", [P, 4 * DH], bf16, kind="ExternalInput").ap()
    w2 = nc.dram_tensor("w2", [P, NH * D], bf16, kind="ExternalInput").ap()
    wd = nc.dram_tensor("wd", [P, NDC * ER], bf16, kind="ExternalInput").ap()
    wu = nc.dram_tensor("wu", [ER, D], bf16, kind="ExternalInput").ap()
    b1 = nc.dram_tensor("b1", [P, NH], f32, kind="ExternalInput").ap()
    b2 = nc.dram_tensor("b2", [P, D], f32, kind="ExternalInput").ap()
    tkp = nc.dram_tensor("tkp", [P, NT * 2], f32, kind="ExternalInput").ap()
    tki = nc.dram_tensor("tki", [P, NT * 4], i32, kind="ExternalInput").ap()
    ident_d = nc.dram_tensor("ident", [P, P], bf16, kind="ExternalInput").ap()
    out = nc.dram_tensor("out", [TC, D], f32, kind="ExternalOutput").ap()

    with tile.TileContext(nc) as tc, ExitStack() as ctx:
        cons = ctx.enter_context(tc.tile_pool(name="cons", bufs=1))
        xtp = ctx.enter_context(tc.tile_pool(name="xtp", bufs=4))
        actp = ctx.enter_context(tc.tile_pool(name="actp", bufs=2))
        outp = ctx.enter_context(tc.tile_pool(name="outp", bufs=4))
        lp = ctx.enter_context(tc.tile_pool(name="lp", bufs=4))
        gtp = ctx.enter_context(tc.tile_pool(name="gtp", bufs=8))
        ps_h = ctx.enter_context(tc.tile_pool(name="ps_h", bufs=4, space="PSUM"))
        ps_o = ctx.enter_context(tc.tile_pool(name="ps_o", bufs=2, space="PSUM"))
        ps_g = ctx.enter_context(tc.tile_pool(name="ps_g", bufs=1, space="PSUM"))
        ps_t = ctx.enter_context(tc.tile_pool(name="ps_t", bufs=1, space="PSUM"))

        # preload the Gelu activation table before the first real activation
        warm_in = cons.tile([1, 8], f32)
        nc.vector.memset(warm_in[:], 0.125)
        warm_out = cons.tile([1, 8], f32)
        nc.scalar.activation(warm_out[:], warm_in[:], AF.Gelu)

        # ---- DMAs in issue order (earlier == higher effective priority).
        # All sources are host-packed so every DMA is one contiguous chunk
        # per partition (cheap descriptor generation on the sync engine).
        ident_sb = cons.tile([P, P], bf16)
        nc.sync.dma_start(ident_sb[:], ident_d)

        xT_tiles = {}

        def dma_xt(g):
            t0g, ngg = groups[g]
            G = ngg * P
            xTt = xtp.tile([P, NDC * G], bf16, tag="xT", name=f"xT{g}")
            nc.sync.dma_start(xTt[:], xt[:, NDC * t0g * P:NDC * (t0g + ngg) * P])
            xT_tiles[g] = xTt

        W1s = cons.tile([P, 4 * DH], bf16)
        nc.sync.dma_start(W1s[:, 0:DH], w1[:, 0:DH])  # h-quarter 0
        dma_xt(0)
        nc.sync.dma_start(W1s[:, DH:2 * DH], w1[:, DH:2 * DH])
        dma_xt(1)
        for q in (2, 3):
            nc.sync.dma_start(W1s[:, q * DH:(q + 1) * DH],
                              w1[:, q * DH:(q + 1) * DH])
        wdr = cons.tile([P, NDC * ER], bf16)
        nc.sync.dma_start(wdr[:], wd)
        wur = cons.tile([ER, D], bf16)
        nc.sync.dma_start(wur[:], wu)
        b1s = cons.tile([P, NH], f32)
        nc.sync.dma_start(b1s[:], b1)
        b2rep = cons.tile([P, D], f32)
        nc.sync.dma_start(b2rep[:], b2)
        W2s = cons.tile([P, NH * D], bf16)
        for j in range(4):
            nc.sync.dma_start(W2s[:, j * 4 * D:(j + 1) * 4 * D],
                              w2[:, j * 4 * D:(j + 1) * 4 * D])
        tp_sb = cons.tile([P, nt * 2], f32)
        nc.sync.dma_start(tp_sb[:], tkp)
        ti_sb = cons.tile([P, nt * 4], i32)
        nc.sync.dma_start(ti_sb[:], tki)

        # ---- routing weights wts[p, e*nt + tile] (DVE) ----
        idxf = cons.tile([P, nt * 2], f32)
        iv = ti_sb.rearrange("p (n k two) -> p n k two", k=2, two=2)
        nc.vector.tensor_copy(
            idxf.rearrange("p (n k one) -> p n k one", k=2, one=1),
            iv[:, :, :, 0:1])
        wts = cons.tile([P, E * nt], f32)
        for e in range(E):
            eq = cons.tile([P, nt * 2], f32, tag="eq", name=f"eq{e}", bufs=2)
            nc.vector.tensor_scalar(eq[:], idxf[:], float(e), None,
                                    op0=OP.is_equal)
            nc.vector.tensor_tensor(eq[:], eq[:], tp_sb[:], op=OP.mult)
            nc.vector.tensor_reduce(wts[:, e * nt:(e + 1) * nt],
                                    eq.rearrange("p (n k) -> p n k", k=2),
                                    axis=mybir.AxisListType.X, op=OP.add)

        # ---- emit helpers ----
        gts = {}  # group -> list of [ER, P] bf16 lhsT tiles for the MM2 close

        def emit_mm1_chunk(g, h, actT):
            t0g, ngg = groups[g]
            G = ngg * P
            q, hh = h // 4, h % 4
            xTt = xT_tiles[g]
            ph = ps_h.tile([P, 512], f32, tag="ph", name=f"ph{g}_{h}")
            base = q * DH + hh * P
            for c in range(NDC):
                nc.tensor.matmul(
                    ph[:, :G],
                    W1s[:, base + c * 512:base + c * 512 + P],
                    xTt[:, c * G:(c + 1) * G],
                    start=(c == 0), stop=(c == NDC - 1))
            nc.scalar.activation(actT[:, h * G:(h + 1) * G], ph[:, :G],
                                 AF.Gelu, bias=b1s[:, h:h + 1], scale=1.0)

        def emit_lora_down(g):
            t0g, ngg = groups[g]
            G = ngg * P
            xTt = xT_tiles[g]
            pga = ps_g.tile([P, GT * ER], f32, tag="pg", name=f"pg{g}")
            for i in range(ngg):
                for c in range(NDC):
                    nc.tensor.matmul(
                        pga[:, i * ER:(i + 1) * ER],
                        xTt[:, c * G + i * P:c * G + (i + 1) * P],
                        wdr[:, c * ER:(c + 1) * ER],
                        start=(c == 0), stop=(c == NDC - 1))
            g2s = []
            for i in range(ngg):
                tt = t0g + i
                g_sb = lp.tile([P, ER], f32, tag="g_sb", name=f"g_sb{tt}")
                nc.scalar.activation(g_sb[:], pga[:, i * ER:(i + 1) * ER],
                                     AF.Gelu)
                g2 = lp.tile([P, ER], bf16, tag="g2", name=f"g2{tt}")
                for e in range(E):
                    nc.vector.tensor_scalar(
                        g2[:, e * R:(e + 1) * R], g_sb[:, e * R:(e + 1) * R],
                        wts[:, e * nt + tt:e * nt + tt + 1], None,
                        op0=OP.mult)
                g2s.append(g2)
            return g2s

        def emit_lora_t(g, g2s):
            t0g, ngg = groups[g]
            pta = ps_t.tile([ER, GT * P], bf16, tag="pt", name=f"pt{g}")
            for i in range(ngg):
                nc.tensor.matmul(pta[:, i * P:(i + 1) * P], g2s[i][:],
                                 ident_sb[:], is_transpose=True)
            lst = []
            for i in range(ngg):
                gt = gtp.tile([ER, P], bf16, tag="gt", name=f"gt{t0g + i}")
                nc.vector.tensor_copy(gt[:], pta[:, i * P:(i + 1) * P])
                lst.append(gt)
            gts[g] = lst

        def emit_mm2_tile(g, i, actT):
            t0g, ngg = groups[g]
            G = ngg * P
            tt = t0g + i
            po = ps_o.tile([P, D], f32, tag="po", name=f"po{tt}")
            for h in range(NH):
                nc.tensor.matmul(
                    po[:],
                    actT[:, h * G + i * P:h * G + (i + 1) * P],
                    W2s[:, h * D:(h + 1) * D],
                    start=(h == 0), stop=False)
            nc.tensor.matmul(po[:], gts[g][i][:], wur[:],
                             start=False, stop=True)
            o_sb = outp.tile([P, D], f32, tag="o_sb", name=f"o_sb{tt}")
            nc.vector.tensor_tensor(o_sb[:], po[:], b2rep[:], op=OP.add)
            nc.sync.dma_start(out[tt * P:(tt + 1) * P, :], o_sb[:])

        # ---- main loop, two-stage software pipeline on the PE:
        # MM1 of group g+1 is interleaved between MM2 tiles of group g, and
        # the LoRA chain for group g+1 also runs under MM2 of group g, so
        # neither a group boundary nor the final group ever stalls the PE.
        actTs = {}

        def make_actT(g):
            t0g, ngg = groups[g]
            a = actp.tile([P, NH * ngg * P], bf16, tag="actT", name=f"actT{g}")
            actTs[g] = a
            return a

        # pipeline fill: MM1(0), with group 0's LoRA folded in at points
        # where its inputs (wd, routing weights) have arrived
        a0 = make_actT(0)
        for h in range(NH):
            emit_mm1_chunk(0, h, a0)
            if h == 4:
                g0_g2s = emit_lora_down(0)
            if h == 10:
                emit_lora_t(0, g0_g2s)

        for g in range(ngrp):
            t0g, ngg = groups[g]
            actT = actTs[g]
            if g + 1 < ngrp:
                nxt = make_actT(g + 1)
                nxt_g2s = emit_lora_down(g + 1)
                emit_mm2_tile(g, 0, actT)
                emit_lora_t(g + 1, nxt_g2s)
                blocks = ((0, 5), (5, 10), (10, 16))
                for bi, i in enumerate(range(1, ngg)):
                    for h in range(*blocks[bi]):
                        emit_mm1_chunk(g + 1, h, nxt)
                    emit_mm2_tile(g, i, actT)
            else:
                for i in range(ngg):
                    emit_mm2_tile(g, i, actT)
            if g + 2 < ngrp:
                dma_xt(g + 2)

    nc.compile()
    return nc


def _get_nc():
    if "nc" not in _CACHE:
        _CACHE["nc"] = _build()
    return _CACHE["nc"]


def _make_in_maps(inputs):
    import ml_dtypes
    bf16 = ml_dtypes.bfloat16

    x = np.asarray(inputs["x"], dtype=np.float32)
    T = x.size // D
    x_flat = x.reshape(T, D)
    W1 = np.asarray(inputs["W1"], dtype=np.float32)
    W2 = np.asarray(inputs["W2"], dtype=np.float32)
    b1 = np.asarray(inputs["b1"], dtype=np.float32)
    b2 = np.asarray(inputs["b2"], dtype=np.float32)
    wdn = np.asarray(inputs["w_down"], dtype=np.float32)
    wup = np.asarray(inputs["w_up"], dtype=np.float32)
    tkp = np.ascontiguousarray(inputs["topk_probs"], dtype=np.float32)
    tki_in = np.asarray(inputs["topk_indices"])

    # SBUF layouts, bf16 (shared by all cores)
    # W1 [D, DH] -> [p, q, c, hh, x]: lhsT slice (c,h=q*4+hh) is contiguous
    w1p = np.ascontiguousarray(
        W1.reshape(NDC, P, 4, 4, P).transpose(1, 2, 0, 3, 4).reshape(P, 4 * DH)
    ).astype(bf16)
    # W2 [DH, D] -> [p, n, d]
    w2p = np.ascontiguousarray(
        W2.reshape(NH, P, D).transpose(1, 0, 2).reshape(P, NH * D)).astype(bf16)
    # w_down [E, D, R] -> [D, E*R] -> [p, c, er]
    wdf = wdn.transpose(1, 0, 2).reshape(D, ER)
    wdp = np.ascontiguousarray(
        wdf.reshape(NDC, P, ER).transpose(1, 0, 2).reshape(P, NDC * ER)
    ).astype(bf16)
    wup_b = np.ascontiguousarray(wup.reshape(ER, D)).astype(bf16)
    b1p = np.ascontiguousarray(b1.reshape(NH, P).T)
    b2p = np.ascontiguousarray(np.broadcast_to(b2[None, :], (P, D)),
                               dtype=np.float32)
    ident = np.eye(P, dtype=np.float32).astype(bf16)

    groups = []
    t0 = 0
    while t0 < NT:
        ng = min(GT, NT - t0)
        groups.append((t0, ng))
        t0 += ng

    in_maps = []
    for c in range(N_CORES):
        sl = slice(c * TC, (c + 1) * TC)
        # x.T in bf16, regrouped per supergroup to [p, (group: c, t)] so the
        # per-group DMA is one contiguous chunk per partition
        xc = x_flat[sl].T.astype(bf16)          # [D, TC] = [(c p), t]
        xr = xc.reshape(NDC, P, TC)
        parts = [
            np.ascontiguousarray(
                xr[:, :, t0g * P:(t0g + ngg) * P].transpose(1, 0, 2)
            ).reshape(P, -1)
            for t0g, ngg in groups
        ]
        xt_c = np.ascontiguousarray(np.concatenate(parts, axis=1))
        # routing tensors packed to [p, n, k] (token tile n, partition p)
        tkp_c = np.ascontiguousarray(
            tkp[sl].reshape(NT, P, 2).transpose(1, 0, 2)).reshape(P, NT * 2)
        tki4 = np.zeros((TC, 4), dtype=np.int32)
        tki4[:, 0] = tki_in[sl, 0]
        tki4[:, 2] = tki_in[sl, 1]
        tki_c = np.ascontiguousarray(
            tki4.reshape(NT, P, 4).transpose(1, 0, 2)).reshape(P, NT * 4)
        in_maps.append(dict(
            xt=xt_c, w1=w1p, w2=w2p, wd=wdp, wu=wup_b, b1=b1p, b2=b2p,
            tkp=tkp_c, tki=tki_c, ident=ident))
    return in_maps


def _ensure_ntff_hook():
    """Register the axon NTFF profile hook if the image's antenv lacks it."""
    import sys
    import types
    try:
        from antenv.axon_hooks import get_axon_ntff_profile_hook  # noqa: F401
        return True
    except ImportError:
        pass
    try:
        from trn_agent_boot.trn_boot import _ntff_profile_via_ctypes
        mod = types.ModuleType("antenv.axon_hooks")
        _hook = [None]
        mod.set_axon_ntff_profile_hook = lambda h: _hook.__setitem__(0, h)
        mod.get_axon_ntff_profile_hook = lambda: _hook[0]
        sys.modules["antenv.axon_hooks"] = mod
        import antenv
        antenv.axon_hooks = mod
        mod.set_axon_ntff_profile_hook(
            _ntff_profile_via_ctypes("/opt/axon/libaxon_pjrt.so"))
        return True
    except Exception:
        return False


def kernel(**inputs):
    from concourse.bass_utils import run_bass_kernel_spmd

    nc = _get_nc()
    in_maps = _make_in_maps(inputs)
    trace = bool(int(os.environ.get("KERNEL_TRACE", "0")))
    if trace and not _ensure_ntff_hook():
        trace = False
    res = run_bass_kernel_spmd(nc, in_maps, list(range(N_CORES)), trace=trace)
    if trace:
        _CACHE["last_result"] = res
    out = np.concatenate([res.results[i]["out"] for i in range(N_CORES)], axis=0)
    return out.reshape(np.asarray(inputs["x"]).shape).astype(np.float32)


# revision 13
# speedup vs baseline: 1.1776x; 1.0083x over previous
"""Trainium2 Bass kernel: ConvNeXt MLP + parallel MoE-LoRA (data-parallel over tokens).

Math per token t (D=512, Dh=2048, E=3 experts, r=8, top-k=2):
    base = gelu(x @ W1 + b1) @ W2 + b2
    g_e  = gelu(x @ w_down[e]) * wts[e, t]          (wts from top-k routing)
    out  = base + sum_e g_e @ w_up[e]

Strategy (per NeuronCore, 8 cores data-parallel on the token dim):
  - all operands are pre-packed on the host into their SBUF layouts in bf16:
    x arrives pre-transposed as xT [D, T_core] so the PE never runs x
    transposes and the DVE never runs casts; weights arrive in matmul-ready
    chunk layouts; b2 arrives pre-replicated [128, 512].
  - tokens tiled 128 at a time; supergroups of 4 tiles (512 tokens) so the
    MM1 moving free dim is 512.
  - MM1: hT[h,t] = W1_chunk.T @ xT (feature-major hidden, 4 PSUM banks so
    the Gelu drain never stalls the accumulation cadence), fused bias+gelu
    on ScalarE into actT (bf16).
  - MM2: out[t,d] accumulates 16 h-chunks (lhsT = actT slices) + the
    MoE-LoRA rank-24 matmul in one PSUM accumulation group; b2 added during
    the PSUM->SBUF drain from the preloaded replicated bias tile.
  - LoRA for supergroup g+1 is computed during MM2 of supergroup g (down
    proj -> gelu -> routing scale -> PE transpose), so the close matmul of
    MM2 never waits on the scalar/vector chain.
  - routing weights wts[e,t] = sum_k probs[t,k]*(idx[t,k]==e) computed on
    device in a small DVE prologue over all tokens at once.
  - W1 is DMA'd in h-quarter pieces so MM1 starts after ~1MB of weight
    traffic; the Gelu activation table is pre-warmed by a dummy activation.
"""

import os
import numpy as np

P = 128
D = 512
DH = 2048
E = 3
R = 8
ER = E * R  # 24
NH = DH // P  # 16
NDC = D // P  # 4 d-chunks
N_CORES = 8
T_FULL = 64 * 28 * 28  # 50176
TC = T_FULL // N_CORES  # 6272
NT = TC // P  # 49 token tiles
GT = 4  # tiles per supergroup

_CACHE = {}


def _build():
    import concourse.bacc as bacc
    import concourse.tile as tile
    import concourse.mybir as mybir
    from contextlib import ExitStack

    f32 = mybir.dt.float32
    bf16 = mybir.dt.bfloat16
    i32 = mybir.dt.int32
    AF = mybir.ActivationFunctionType
    OP = mybir.AluOpType

    nt = NT
    groups = []
    t0 = 0
    while t0 < nt:
        ng = min(GT, nt - t0)
        groups.append((t0, ng))
        t0 += ng
    ngrp = len(groups)

    nc = bacc.Bacc("TRN2", target_bir_lowering=False, debug=False,
                   num_devices=N_CORES)

    xt = nc.dram_tensor("xt", [P, NDC * TC], bf16, kind="ExternalInput").ap()
    w1 = nc.dram_tensor("w1", [P, 4 * DH], bf16, kind="ExternalInput").ap()
    w2 = nc.dram_tensor("w2", [P, NH * D], bf16, kind="ExternalInput").ap()
    wd = nc.dram_tensor("wd", [P, NDC * ER], bf16, kind="ExternalInput").ap()
    wu = nc.dram_tensor("wu", [ER, D], bf16, kind="ExternalInput").ap()
    b1 = nc.dram_tensor("b1", [P, NH], f32, kind="ExternalInput").ap()
    b2 = nc.dram_tensor("b2", [P, D], f32, kind="ExternalInput").ap()
    tkp = nc.dram_tensor("tkp", [P, NT * 2], f32, kind="ExternalInput").ap()
    tki = nc.dram_tensor("tki", [P, NT * 4], i32, kind="ExternalInput").ap()
    ident_d = nc.dram_tensor("ident", [P, P], bf16, kind="ExternalInput").ap()
    out = nc.dram_tensor("out", [TC, D], f32, kind="ExternalOutput").ap()

    with tile.TileContext(nc) as tc, ExitStack() as ctx:
        cons = ctx.enter_context(tc.tile_pool(name="cons", bufs=1))
        xtp = ctx.enter_context(tc.tile_pool(name="xtp", bufs=4))
        actp = ctx.enter_context(tc.tile_pool(name="actp", bufs=2))
        outp = ctx.enter_context(tc.tile_pool(name="outp", bufs=4))
        lp = ctx.enter_context(tc.tile_pool(name="lp", bufs=4))
        gtp = ctx.enter_context(tc.tile_pool(name="gtp", bufs=8))
        ps_h = ctx.enter_context(tc.tile_pool(name="ps_h", bufs=4, space="PSUM"))
        ps_o = ctx.enter_context(tc.tile_pool(name="ps_o", bufs=2, space="PSUM"))
        ps_g = ctx.enter_context(tc.tile_pool(name="ps_g", bufs=1, space="PSUM"))
        ps_t = ctx.enter_context(tc.tile_pool(name="ps_t", bufs=1, space="PSUM"))

        # preload the Gelu activation table before the first real activation
        warm_in = cons.tile([1, 8], f32)
        nc.vector.memset(warm_in[:], 0.125)
        warm_out = cons.tile([1, 8], f32)
        nc.scalar.activation(warm_out[:], warm_in[:], AF.Gelu)
        # dummy matmul fodder: the PE clock ramps ~6us after sustained
        # activity starts, so burn the DMA wait on throwaway matmuls to
        # trigger the ramp before the real data lands
        dum_in = cons.tile([P, 512], bf16)
        nc.vector.memset(dum_in[:], 0.0)

        # ---- DMAs in issue order (earlier == higher effective priority).
        # All sources are host-packed so every DMA is one contiguous chunk
        # per partition (cheap descriptor generation on the sync engine).
        ident_sb = cons.tile([P, P], bf16)
        nc.sync.dma_start(ident_sb[:], ident_d)

        xT_tiles = {}

        def dma_xt(g):
            t0g, ngg = groups[g]
            G = ngg * P
            xTt = xtp.tile([P, NDC * G], bf16, tag="xT", name=f"xT{g}")
            nc.sync.dma_start(xTt[:], xt[:, NDC * t0g * P:NDC * (t0g + ngg) * P])
            xT_tiles[g] = xTt

        # first-needed data in small pieces so MM1 h-chunk 0 starts earliest:
        # W1 quarter 0 in four hh-pieces, xT group 0 in four c-pieces
        W1s = cons.tile([P, 4 * DH], bf16)
        xT0 = xtp.tile([P, NDC * GT * P], bf16, tag="xT", name="xT0")
        xT_tiles[0] = xT0
        b1s = cons.tile([P, NH], f32)
        wdr = cons.tile([P, NDC * ER], bf16)
        nc.sync.dma_start(W1s[:, 0:512], w1[:, 0:512])        # q0 hh0
        nc.sync.dma_start(xT0[:, 0:512], xt[:, 0:512])        # g0 c0
        nc.sync.dma_start(b1s[:], b1)
        nc.sync.dma_start(xT0[:, 512:1024], xt[:, 512:1024])  # g0 c1
        nc.sync.dma_start(W1s[:, 512:1024], w1[:, 512:1024])  # q0 hh1
        nc.sync.dma_start(xT0[:, 1024:1536], xt[:, 1024:1536])  # g0 c2
        nc.sync.dma_start(xT0[:, 1536:2048], xt[:, 1536:2048])  # g0 c3
        nc.sync.dma_start(W1s[:, 1024:2048], w1[:, 1024:2048])  # q0 hh2-3
        nc.sync.dma_start(wdr[:], wd)
        dma_xt(1)
        for q in (1, 2, 3):
            nc.sync.dma_start(W1s[:, q * DH:(q + 1) * DH],
                              w1[:, q * DH:(q + 1) * DH])
        wur = cons.tile([ER, D], bf16)
        nc.sync.dma_start(wur[:], wu)
        b2rep = cons.tile([P, D], f32)
        nc.sync.dma_start(b2rep[:], b2)
        W2s = cons.tile([P, NH * D], bf16)
        for j in range(4):
            nc.sync.dma_start(W2s[:, j * 4 * D:(j + 1) * 4 * D],
                              w2[:, j * 4 * D:(j + 1) * 4 * D])
        tp_sb = cons.tile([P, nt * 2], f32)
        nc.sync.dma_start(tp_sb[:], tkp)
        ti_sb = cons.tile([P, nt * 4], i32)
        nc.sync.dma_start(ti_sb[:], tki)

        # ---- routing weights wts[p, e*nt + tile] (DVE) ----
        idxf = cons.tile([P, nt * 2], f32)
        iv = ti_sb.rearrange("p (n k two) -> p n k two", k=2, two=2)
        nc.vector.tensor_copy(
            idxf.rearrange("p (n k one) -> p n k one", k=2, one=1),
            iv[:, :, :, 0:1])
        wts = cons.tile([P, E * nt], f32)
        for e in range(E):
            eq = cons.tile([P, nt * 2], f32, tag="eq", name=f"eq{e}", bufs=2)
            nc.vector.tensor_scalar(eq[:], idxf[:], float(e), None,
                                    op0=OP.is_equal)
            nc.vector.tensor_tensor(eq[:], eq[:], tp_sb[:], op=OP.mult)
            nc.vector.tensor_reduce(wts[:, e * nt:(e + 1) * nt],
                                    eq.rearrange("p (n k) -> p n k", k=2),
                                    axis=mybir.AxisListType.X, op=OP.add)

        # ---- emit helpers ----
        gts = {}  # group -> list of [ER, P] bf16 lhsT tiles for the MM2 close

        def emit_mm1_chunk(g, h, actT):
            t0g, ngg = groups[g]
            G = ngg * P
            q, hh = h // 4, h % 4
            xTt = xT_tiles[g]
            ph = ps_h.tile([P, 512], f32, tag="ph", name=f"ph{g}_{h}")
            base = q * DH + hh * 512
            for c in range(NDC):
                nc.tensor.matmul(
                    ph[:, :G],
                    W1s[:, base + c * P:base + (c + 1) * P],
                    xTt[:, c * G:(c + 1) * G],
                    start=(c == 0), stop=(c == NDC - 1))
            nc.scalar.activation(actT[:, h * G:(h + 1) * G], ph[:, :G],
                                 AF.Gelu, bias=b1s[:, h:h + 1], scale=1.0)

        def emit_lora_down(g):
            t0g, ngg = groups[g]
            G = ngg * P
            xTt = xT_tiles[g]
            pga = ps_g.tile([P, GT * ER], f32, tag="pg", name=f"pg{g}")
            for i in range(ngg):
                for c in range(NDC):
                    nc.tensor.matmul(
                        pga[:, i * ER:(i + 1) * ER],
                        xTt[:, c * G + i * P:c * G + (i + 1) * P],
                        wdr[:, c * ER:(c + 1) * ER],
                        start=(c == 0), stop=(c == NDC - 1))
            g2s = []
            for i in range(ngg):
                tt = t0g + i
                g_sb = lp.tile([P, ER], f32, tag="g_sb", name=f"g_sb{tt}")
                nc.scalar.activation(g_sb[:], pga[:, i * ER:(i + 1) * ER],
                                     AF.Gelu)
                g2 = lp.tile([P, ER], bf16, tag="g2", name=f"g2{tt}")
                for e in range(E):
                    nc.vector.tensor_scalar(
                        g2[:, e * R:(e + 1) * R], g_sb[:, e * R:(e + 1) * R],
                        wts[:, e * nt + tt:e * nt + tt + 1], None,
                        op0=OP.mult)
                g2s.append(g2)
            return g2s

        def emit_lora_t(g, g2s):
            t0g, ngg = groups[g]
            pta = ps_t.tile([ER, GT * P], bf16, tag="pt", name=f"pt{g}")
            for i in range(ngg):
                nc.tensor.matmul(pta[:, i * P:(i + 1) * P], g2s[i][:],
                                 ident_sb[:], is_transpose=True)
            lst = []
            for i in range(ngg):
                gt = gtp.tile([ER, P], bf16, tag="gt", name=f"gt{t0g + i}")
                nc.vector.tensor_copy(gt[:], pta[:, i * P:(i + 1) * P])
                lst.append(gt)
            gts[g] = lst

        def emit_mm2_tile(g, i, actT):
            t0g, ngg = groups[g]
            G = ngg * P
            tt = t0g + i
            po = ps_o.tile([P, D], f32, tag="po", name=f"po{tt}")
            for h in range(NH):
                nc.tensor.matmul(
                    po[:],
                    actT[:, h * G + i * P:h * G + (i + 1) * P],
                    W2s[:, h * D:(h + 1) * D],
                    start=(h == 0), stop=False)
            nc.tensor.matmul(po[:], gts[g][i][:], wur[:],
                             start=False, stop=True)
            o_sb = outp.tile([P, D], f32, tag="o_sb", name=f"o_sb{tt}")
            nc.vector.tensor_tensor(o_sb[:], po[:], b2rep[:], op=OP.add)
            nc.sync.dma_start(out[tt * P:(tt + 1) * P, :], o_sb[:])

        # ---- main loop, two-stage software pipeline on the PE:
        # MM1 of group g+1 is interleaved between MM2 tiles of group g, and
        # the LoRA chain for group g+1 also runs under MM2 of group g, so
        # neither a group boundary nor the final group ever stalls the PE.
        actTs = {}

        def make_actT(g):
            t0g, ngg = groups[g]
            a = actp.tile([P, NH * ngg * P], bf16, tag="actT", name=f"actT{g}")
            actTs[g] = a
            return a

        # PE warm-up: throwaway matmuls while the first DMAs land, so the
        # clock ramp (triggered by activity) completes before real work
        for k in range(5):
            pd = ps_o.tile([P, D], f32, tag="po", name=f"dum{k}")
            nc.tensor.matmul(pd[:], dum_in[:, 0:P], dum_in[:],
                             start=True, stop=True)

        # pipeline fill: MM1(0), with group 0's LoRA folded in at points
        # where its inputs (wd, routing weights) have arrived
        a0 = make_actT(0)
        for h in range(NH):
            emit_mm1_chunk(0, h, a0)
            if h == 6:
                g0_g2s = emit_lora_down(0)
            if h == 12:
                emit_lora_t(0, g0_g2s)

        for g in range(ngrp):
            t0g, ngg = groups[g]
            actT = actTs[g]
            if g + 1 < ngrp:
                nxt = make_actT(g + 1)
                nxt_g2s = emit_lora_down(g + 1)
                emit_mm2_tile(g, 0, actT)
                emit_lora_t(g + 1, nxt_g2s)
                blocks = ((0, 5), (5, 10), (10, 16))
                for bi, i in enumerate(range(1, ngg)):
                    for h in range(*blocks[bi]):
                        emit_mm1_chunk(g + 1, h, nxt)
                    emit_mm2_tile(g, i, actT)
            else:
                for i in range(ngg):
                    emit_mm2_tile(g, i, actT)
            if g + 2 < ngrp:
                dma_xt(g + 2)

    nc.compile()
    return nc


def _get_nc():
    if "nc" not in _CACHE:
        _CACHE["nc"] = _build()
    return _CACHE["nc"]


def _make_in_maps(inputs):
    import ml_dtypes
    bf16 = ml_dtypes.bfloat16

    x = np.asarray(inputs["x"], dtype=np.float32)
    T = x.size // D
    x_flat = x.reshape(T, D)
    W1 = np.asarray(inputs["W1"], dtype=np.float32)
    W2 = np.asarray(inputs["W2"], dtype=np.float32)
    b1 = np.asarray(inputs["b1"], dtype=np.float32)
    b2 = np.asarray(inputs["b2"], dtype=np.float32)
    wdn = np.asarray(inputs["w_down"], dtype=np.float32)
    wup = np.asarray(inputs["w_up"], dtype=np.float32)
    tkp = np.ascontiguousarray(inputs["topk_probs"], dtype=np.float32)
    tki_in = np.asarray(inputs["topk_indices"])

    # SBUF layouts, bf16 (shared by all cores)
    # W1 [D, DH] -> [p, q, hh, c, x]: hh-major so the first DMA piece covers
    # the first MM1 h-chunks
    w1p = np.ascontiguousarray(
        W1.reshape(NDC, P, 4, 4, P).transpose(1, 2, 3, 0, 4).reshape(P, 4 * DH)
    ).astype(bf16)
    # W2 [DH, D] -> [p, n, d]
    w2p = np.ascontiguousarray(
        W2.reshape(NH, P, D).transpose(1, 0, 2).reshape(P, NH * D)).astype(bf16)
    # w_down [E, D, R] -> [D, E*R] -> [p, c, er]
    wdf = wdn.transpose(1, 0, 2).reshape(D, ER)
    wdp = np.ascontiguousarray(
        wdf.reshape(NDC, P, ER).transpose(1, 0, 2).reshape(P, NDC * ER)
    ).astype(bf16)
    wup_b = np.ascontiguousarray(wup.reshape(ER, D)).astype(bf16)
    b1p = np.ascontiguousarray(b1.reshape(NH, P).T)
    b2p = np.ascontiguousarray(np.broadcast_to(b2[None, :], (P, D)),
                               dtype=np.float32)
    ident = np.eye(P, dtype=np.float32).astype(bf16)

    groups = []
    t0 = 0
    while t0 < NT:
        ng = min(GT, NT - t0)
        groups.append((t0, ng))
        t0 += ng

    in_maps = []
    for c in range(N_CORES):
        sl = slice(c * TC, (c + 1) * TC)
        # x.T in bf16, regrouped per supergroup to [p, (group: c, t)] so the
        # per-group DMA is one contiguous chunk per partition
        xc = x_flat[sl].T.astype(bf16)          # [D, TC] = [(c p), t]
        xr = xc.reshape(NDC, P, TC)
        parts = [
            np.ascontiguousarray(
                xr[:, :, t0g * P:(t0g + ngg) * P].transpose(1, 0, 2)
            ).reshape(P, -1)
            for t0g, ngg in groups
        ]
        xt_c = np.ascontiguousarray(np.concatenate(parts, axis=1))
        # routing tensors packed to [p, n, k] (token tile n, partition p)
        tkp_c = np.ascontiguousarray(
            tkp[sl].reshape(NT, P, 2).transpose(1, 0, 2)).reshape(P, NT * 2)
        tki4 = np.zeros((TC, 4), dtype=np.int32)
        tki4[:, 0] = tki_in[sl, 0]
        tki4[:, 2] = tki_in[sl, 1]
        tki_c = np.ascontiguousarray(
            tki4.reshape(NT, P, 4).transpose(1, 0, 2)).reshape(P, NT * 4)
        in_maps.append(dict(
            xt=xt_c, w1=w1p, w2=w2p, wd=wdp, wu=wup_b, b1=b1p, b2=b2p,
            tkp=tkp_c, tki=tki_c, ident=ident))
    return in_maps


def _ensure_ntff_hook():
    """Register the axon NTFF profile hook if the image's antenv lacks it."""
    import sys
    import types
    try:
        from antenv.axon_hooks import get_axon_ntff_profile_hook  # noqa: F401
        return True
    except ImportError:
        pass
    try:
        from trn_agent_boot.trn_boot import _ntff_profile_via_ctypes
        mod = types.ModuleType("antenv.axon_hooks")
        _hook = [None]
        mod.set_axon_ntff_profile_hook = lambda h: _hook.__setitem__(0, h)
        mod.get_axon_ntff_profile_hook = lambda: _hook[0]
        sys.modules["antenv.axon_hooks"] = mod
        import antenv
        antenv.axon_hooks = mod
        mod.set_axon_ntff_profile_hook(
            _ntff_profile_via_ctypes("/opt/axon/libaxon_pjrt.so"))
        return True
    except Exception:
        return False


def kernel(**inputs):
    from concourse.bass_utils import run_bass_kernel_spmd

    nc = _get_nc()
    in_maps = _make_in_maps(inputs)
    trace = bool(int(os.environ.get("KERNEL_TRACE", "0")))
    if trace and not _ensure_ntff_hook():
        trace = False
    res = run_bass_kernel_spmd(nc, in_maps, list(range(N_CORES)), trace=trace)
    if trace:
        _CACHE["last_result"] = res
    out = np.concatenate([res.results[i]["out"] for i in range(N_CORES)], axis=0)
    return out.reshape(np.asarray(inputs["x"]).shape).astype(np.float32)


# revision 24
# speedup vs baseline: 1.1867x; 1.0077x over previous
"""Trainium2 Bass kernel: ConvNeXt MLP + parallel MoE-LoRA (data-parallel over tokens).

Math per token t (D=512, Dh=2048, E=3 experts, r=8, top-k=2):
    base = gelu(x @ W1 + b1) @ W2 + b2
    g_e  = gelu(x @ w_down[e]) * wts[e, t]          (wts from top-k routing)
    out  = base + sum_e g_e @ w_up[e]

Strategy (per NeuronCore, 8 cores data-parallel on the token dim):
  - all operands are pre-packed on the host into their SBUF layouts in bf16:
    x arrives pre-transposed as xT [D, T_core] so the PE never runs x
    transposes and the DVE never runs casts; weights arrive in matmul-ready
    chunk layouts; b2 arrives pre-replicated [128, 512].
  - tokens tiled 128 at a time; supergroups of 4 tiles (512 tokens) so the
    MM1 moving free dim is 512.
  - MM1: hT[h,t] = W1_chunk.T @ xT (feature-major hidden, 4 PSUM banks so
    the Gelu drain never stalls the accumulation cadence), fused bias+gelu
    on ScalarE into actT (bf16).
  - MM2: out[t,d] accumulates 16 h-chunks (lhsT = actT slices) + the
    MoE-LoRA rank-24 matmul in one PSUM accumulation group; b2 added during
    the PSUM->SBUF drain from the preloaded replicated bias tile.
  - LoRA for supergroup g+1 is computed during MM2 of supergroup g (down
    proj -> gelu -> routing scale -> PE transpose), so the close matmul of
    MM2 never waits on the scalar/vector chain.
  - routing weights wts[e,t] = sum_k probs[t,k]*(idx[t,k]==e) computed on
    device in a small DVE prologue over all tokens at once.
  - W1 is DMA'd in h-quarter pieces so MM1 starts after ~1MB of weight
    traffic; the Gelu activation table is pre-warmed by a dummy activation.
"""

import os
import numpy as np

P = 128
D = 512
DH = 2048
E = 3
R = 8
ER = E * R  # 24
NH = DH // P  # 16
NDC = D // P  # 4 d-chunks
N_CORES = 8
T_FULL = 64 * 28 * 28  # 50176
TC = T_FULL // N_CORES  # 6272
NT = TC // P  # 49 token tiles
GT = 4  # tiles per supergroup

_CACHE = {}


def _build():
    import concourse.bacc as bacc
    import concourse.tile as tile
    import concourse.mybir as mybir
    from contextlib import ExitStack

    f32 = mybir.dt.float32
    bf16 = mybir.dt.bfloat16
    i32 = mybir.dt.int32
    AF = mybir.ActivationFunctionType
    OP = mybir.AluOpType

    nt = NT
    groups = []
    t0 = 0
    while t0 < nt:
        ng = min(GT, nt - t0)
        groups.append((t0, ng))
        t0 += ng
    ngrp = len(groups)

    nc = bacc.Bacc("TRN2", target_bir_lowering=False, debug=False,
                   num_devices=N_CORES)

    xt = nc.dram_tensor("xt", [P, NDC * TC], bf16, kind="ExternalInput").ap()
    w1 = nc.dram_tensor("w1", [P, 4 * DH], bf16, kind="ExternalInput").ap()
    w2 = nc.dram_tensor("w2", [P, NH * D], bf16, kind="ExternalInput").ap()
    wd = nc.dram_tensor("wd", [P, NDC * ER], bf16, kind="ExternalInput").ap()
    wu = nc.dram_tensor("wu", [P, D], bf16, kind="ExternalInput").ap()
    b1 = nc.dram_tensor("b1", [P, NH], f32, kind="ExternalInput").ap()
    b2 = nc.dram_tensor("b2", [P, D], f32, kind="ExternalInput").ap()
    tkp = nc.dram_tensor("tkp", [P, NT * 2], f32, kind="ExternalInput").ap()
    tki = nc.dram_tensor("tki", [P, NT * 4], i32, kind="ExternalInput").ap()
    ident_d = nc.dram_tensor("ident", [P, P], bf16, kind="ExternalInput").ap()
    out = nc.dram_tensor("out", [TC, D], f32, kind="ExternalOutput").ap()

    with tile.TileContext(nc) as tc, ExitStack() as ctx:
        cons = ctx.enter_context(tc.tile_pool(name="cons", bufs=1))
        xtp = ctx.enter_context(tc.tile_pool(name="xtp", bufs=4))
        actp = ctx.enter_context(tc.tile_pool(name="actp", bufs=2))
        outp = ctx.enter_context(tc.tile_pool(name="outp", bufs=4))
        lp = ctx.enter_context(tc.tile_pool(name="lp", bufs=4))
        gtp = ctx.enter_context(tc.tile_pool(name="gtp", bufs=8))
        ps_h = ctx.enter_context(tc.tile_pool(name="ps_h", bufs=3, space="PSUM"))
        ps_o = ctx.enter_context(tc.tile_pool(name="ps_o", bufs=2, space="PSUM"))
        ps_g = ctx.enter_context(tc.tile_pool(name="ps_g", bufs=1, space="PSUM"))
        ps_t = ctx.enter_context(tc.tile_pool(name="ps_t", bufs=1, space="PSUM"))

        # preload the Gelu activation table before the first real activation
        warm_in = cons.tile([1, 8], f32)
        nc.vector.memset(warm_in[:], 0.125)
        warm_out = cons.tile([1, 8], f32)
        nc.scalar.activation(warm_out[:], warm_in[:], AF.Gelu)
        # dummy matmul fodder: the PE clock ramps ~6us after sustained
        # activity starts, so burn the DMA wait on throwaway matmuls to
        # trigger the ramp before the real data lands
        dum_in = cons.tile([P, 512], bf16)
        nc.vector.memset(dum_in[:], 0.0)

        # ---- DMAs in issue order (earlier == higher effective priority).
        # All sources are host-packed so every DMA is one contiguous chunk
        # per partition (cheap descriptor generation on the sync engine).
        ident_sb = cons.tile([P, P], bf16)
        nc.sync.dma_start(ident_sb[:], ident_d)

        xT_tiles = {}

        def dma_xt(g):
            t0g, ngg = groups[g]
            G = ngg * P
            xTt = xtp.tile([P, NDC * G], bf16, tag="xT", name=f"xT{g}")
            nc.sync.dma_start(xTt[:], xt[:, NDC * t0g * P:NDC * (t0g + ngg) * P])
            xT_tiles[g] = xTt

        # first-needed data in small pieces so MM1 h-chunk 0 starts earliest:
        # W1 quarter 0 in four hh-pieces, xT group 0 in four c-pieces
        W1s = cons.tile([P, 4 * DH], bf16)
        xT0 = xtp.tile([P, NDC * GT * P], bf16, tag="xT", name="xT0")
        xT_tiles[0] = xT0
        b1s = cons.tile([P, NH], f32)
        wdr = cons.tile([P, NDC * ER], bf16)
        nc.sync.dma_start(W1s[:, 0:512], w1[:, 0:512])        # q0 hh0
        nc.sync.dma_start(xT0[:, 0:512], xt[:, 0:512])        # g0 c0
        nc.sync.dma_start(b1s[:], b1)
        nc.sync.dma_start(xT0[:, 512:1024], xt[:, 512:1024])  # g0 c1
        nc.sync.dma_start(W1s[:, 512:1024], w1[:, 512:1024])  # q0 hh1
        nc.sync.dma_start(xT0[:, 1024:1536], xt[:, 1024:1536])  # g0 c2
        nc.sync.dma_start(xT0[:, 1536:2048], xt[:, 1536:2048])  # g0 c3
        nc.sync.dma_start(W1s[:, 1024:2048], w1[:, 1024:2048])  # q0 hh2-3
        nc.sync.dma_start(wdr[:], wd)
        for q in (1, 2, 3):
            nc.sync.dma_start(W1s[:, q * DH:(q + 1) * DH],
                              w1[:, q * DH:(q + 1) * DH])
        tp_sb = cons.tile([P, nt * 2], f32)
        nc.sync.dma_start(tp_sb[:], tkp)
        ti_sb = cons.tile([P, nt * 4], i32)
        nc.sync.dma_start(ti_sb[:], tki)
        dma_xt(1)
        W2s = cons.tile([P, NH * D], bf16)
        for j in range(4):
            nc.sync.dma_start(W2s[:, j * 4 * D:(j + 1) * 4 * D],
                              w2[:, j * 4 * D:(j + 1) * 4 * D])
        # w_up replicated at partition offsets 0/32/64/96 so each tile's
        # close matmul has lhsT/rhs at the same (aligned) base partition
        wur = cons.tile([P, D], bf16)
        nc.sync.dma_start(wur[:], wu)
        b2rep = cons.tile([P, D], f32)
        nc.sync.dma_start(b2rep[:], b2)

        # ---- routing weights wts[p, e*nt + tile] (DVE) ----
        idxf = cons.tile([P, nt * 2], f32)
        iv = ti_sb.rearrange("p (n k two) -> p n k two", k=2, two=2)
        nc.vector.tensor_copy(
            idxf.rearrange("p (n k one) -> p n k one", k=2, one=1),
            iv[:, :, :, 0:1])
        wts = cons.tile([P, E * nt], f32)
        for e in range(E):
            eq = cons.tile([P, nt * 2], f32, tag="eq", name=f"eq{e}", bufs=2)
            nc.vector.tensor_scalar(eq[:], idxf[:], float(e), None,
                                    op0=OP.is_equal)
            nc.vector.tensor_tensor(eq[:], eq[:], tp_sb[:], op=OP.mult)
            nc.vector.tensor_reduce(wts[:, e * nt:(e + 1) * nt],
                                    eq.rearrange("p (n k) -> p n k", k=2),
                                    axis=mybir.AxisListType.X, op=OP.add)

        # ---- emit helpers ----
        gts = {}  # group -> list of [ER, P] bf16 lhsT tiles for the MM2 close

        def emit_mm1_chunk(g, h, actT):
            t0g, ngg = groups[g]
            G = ngg * P
            q, hh = h // 4, h % 4
            xTt = xT_tiles[g]
            ph = ps_h.tile([P, 512], f32, tag="ph", name=f"ph{g}_{h}")
            base = q * DH + hh * 512
            for c in range(NDC):
                nc.tensor.matmul(
                    ph[:, :G],
                    W1s[:, base + c * P:base + (c + 1) * P],
                    xTt[:, c * G:(c + 1) * G],
                    start=(c == 0), stop=(c == NDC - 1))
            nc.scalar.activation(actT[:, h * G:(h + 1) * G], ph[:, :G],
                                 AF.Gelu, bias=b1s[:, h:h + 1], scale=1.0)

        def emit_lora_down(g):
            t0g, ngg = groups[g]
            G = ngg * P
            xTt = xT_tiles[g]
            pga = ps_g.tile([P, GT * ER], f32, tag="pg", name=f"pg{g}")
            for i in range(ngg):
                for c in range(NDC):
                    nc.tensor.matmul(
                        pga[:, i * ER:(i + 1) * ER],
                        xTt[:, c * G + i * P:c * G + (i + 1) * P],
                        wdr[:, c * ER:(c + 1) * ER],
                        start=(c == 0), stop=(c == NDC - 1))
            g_sba = lp.tile([P, ngg * ER], f32, tag="g_sb", name=f"g_sb{g}")
            nc.scalar.activation(g_sba[:], pga[:, :ngg * ER], AF.Gelu)
            # g2 blocks padded to a 32-column stride so the transposed rows
            # land at base partitions 0/32/64/96 (PE tile-position rule)
            g2a = lp.tile([P, ngg * 32], bf16, tag="g2", name=f"g2{g}")
            for i in range(ngg):
                tt = t0g + i
                for e in range(E):
                    nc.vector.tensor_scalar(
                        g2a[:, i * 32 + e * R:i * 32 + (e + 1) * R],
                        g_sba[:, i * ER + e * R:i * ER + (e + 1) * R],
                        wts[:, e * nt + tt:e * nt + tt + 1], None,
                        op0=OP.mult)
            return g2a

        def emit_lora_t(g, g2a):
            # transpose two tiles at a time: [128, 64] -> [64, 128], so the
            # per-tile rows sit at base partition 0/32 (PE quadrant rule)
            t0g, ngg = groups[g]
            lst = []
            for h0 in range(0, ngg, 2):
                w = min(2, ngg - h0) * 32
                pta = ps_t.tile([w, P], bf16, tag="pt", name=f"pt{g}_{h0}",
                                bufs=2)
                nc.tensor.matmul(pta[:], g2a[:, h0 * 32:h0 * 32 + w],
                                 ident_sb[:], is_transpose=True)
                gta = gtp.tile([w, P], bf16, tag="gt", name=f"gt{g}_{h0}")
                nc.vector.tensor_copy(gta[:], pta[:])
                for j in range(w // 32):
                    lst.append(gta[j * 32:j * 32 + ER, :])
            gts[g] = lst

        def emit_mm2_tile(g, i, actT):
            t0g, ngg = groups[g]
            G = ngg * P
            tt = t0g + i
            po = ps_o.tile([P, D], f32, tag="po", name=f"po{tt}")
            for h in range(NH):
                nc.tensor.matmul(
                    po[:],
                    actT[:, h * G + i * P:h * G + (i + 1) * P],
                    W2s[:, h * D:(h + 1) * D],
                    start=(h == 0), stop=False)
            nc.tensor.matmul(po[:], gts[g][i],
                             wur[(i % 2) * 32:(i % 2) * 32 + ER, :],
                             start=False, stop=True)
            o_sb = outp.tile([P, D], f32, tag="o_sb", name=f"o_sb{tt}")
            nc.vector.tensor_tensor(o_sb[:], po[:], b2rep[:], op=OP.add)
            nc.sync.dma_start(out[tt * P:(tt + 1) * P, :], o_sb[:])

        # ---- main loop, two-stage software pipeline on the PE:
        # MM1 of group g+1 is interleaved between MM2 tiles of group g, and
        # the LoRA chain for group g+1 also runs under MM2 of group g, so
        # neither a group boundary nor the final group ever stalls the PE.
        actTs = {}

        def make_actT(g):
            t0g, ngg = groups[g]
            a = actp.tile([P, NH * ngg * P], bf16, tag="actT", name=f"actT{g}")
            actTs[g] = a
            return a

        # PE warm-up: throwaway matmuls while the first DMAs land, so the
        # clock ramp (triggered by activity) completes before real work
        for k in range(5):
            pd = ps_o.tile([P, D], f32, tag="po", name=f"dum{k}")
            nc.tensor.matmul(pd[:], dum_in[:, 0:P], dum_in[:],
                             start=True, stop=True)

        # pipeline fill: MM1(0), with group 0's LoRA folded in at points
        # where its inputs (wd, routing weights) have arrived
        a0 = make_actT(0)
        for h in range(NH):
            emit_mm1_chunk(0, h, a0)
            if h == 6:
                g0_g2s = emit_lora_down(0)
            if h == 12:
                emit_lora_t(0, g0_g2s)

        for g in range(ngrp):
            t0g, ngg = groups[g]
            actT = actTs[g]
            if g + 1 < ngrp:
                nxt = make_actT(g + 1)
                nxt_g2s = emit_lora_down(g + 1)
                emit_mm2_tile(g, 0, actT)
                emit_lora_t(g + 1, nxt_g2s)
                blocks = ((0, 5), (5, 10), (10, 16))
                for bi, i in enumerate(range(1, ngg)):
                    for h in range(*blocks[bi]):
                        emit_mm1_chunk(g + 1, h, nxt)
                    emit_mm2_tile(g, i, actT)
            else:
                for i in range(ngg):
                    emit_mm2_tile(g, i, actT)
            if g + 2 < ngrp:
                dma_xt(g + 2)

    nc.compile()
    return nc


def _get_nc():
    if "nc" not in _CACHE:
        _CACHE["nc"] = _build()
    return _CACHE["nc"]


def _make_in_maps(inputs):
    import ml_dtypes
    bf16 = ml_dtypes.bfloat16

    x = np.asarray(inputs["x"], dtype=np.float32)
    T = x.size // D
    x_flat = x.reshape(T, D)
    W1 = np.asarray(inputs["W1"], dtype=np.float32)
    W2 = np.asarray(inputs["W2"], dtype=np.float32)
    b1 = np.asarray(inputs["b1"], dtype=np.float32)
    b2 = np.asarray(inputs["b2"], dtype=np.float32)
    wdn = np.asarray(inputs["w_down"], dtype=np.float32)
    wup = np.asarray(inputs["w_up"], dtype=np.float32)
    tkp = np.ascontiguousarray(inputs["topk_probs"], dtype=np.float32)
    tki_in = np.asarray(inputs["topk_indices"])

    # SBUF layouts, bf16 (shared by all cores)
    # W1 [D, DH] -> [p, q, hh, c, x]: hh-major so the first DMA piece covers
    # the first MM1 h-chunks
    w1p = np.ascontiguousarray(
        W1.reshape(NDC, P, 4, 4, P).transpose(1, 2, 3, 0, 4).reshape(P, 4 * DH)
    ).astype(bf16)
    # W2 [DH, D] -> [p, n, d]
    w2p = np.ascontiguousarray(
        W2.reshape(NH, P, D).transpose(1, 0, 2).reshape(P, NH * D)).astype(bf16)
    # w_down [E, D, R] -> [D, E*R] -> [p, c, er]
    wdf = wdn.transpose(1, 0, 2).reshape(D, ER)
    wdp = np.ascontiguousarray(
        wdf.reshape(NDC, P, ER).transpose(1, 0, 2).reshape(P, NDC * ER)
    ).astype(bf16)
    wup_b = np.zeros((P, D), dtype=bf16)
    for i in range(GT):
        wup_b[i * 32:i * 32 + ER, :] = wup.reshape(ER, D).astype(bf16)
    b1p = np.ascontiguousarray(b1.reshape(NH, P).T)
    b2p = np.ascontiguousarray(np.broadcast_to(b2[None, :], (P, D)),
                               dtype=np.float32)
    ident = np.eye(P, dtype=np.float32).astype(bf16)

    groups = []
    t0 = 0
    while t0 < NT:
        ng = min(GT, NT - t0)
        groups.append((t0, ng))
        t0 += ng

    in_maps = []
    for c in range(N_CORES):
        sl = slice(c * TC, (c + 1) * TC)
        # x.T in bf16, regrouped per supergroup to [p, (group: c, t)] so the
        # per-group DMA is one contiguous chunk per partition
        xc = x_flat[sl].T.astype(bf16)          # [D, TC] = [(c p), t]
        xr = xc.reshape(NDC, P, TC)
        parts = [
            np.ascontiguousarray(
                xr[:, :, t0g * P:(t0g + ngg) * P].transpose(1, 0, 2)
            ).reshape(P, -1)
            for t0g, ngg in groups
        ]
        xt_c = np.ascontiguousarray(np.concatenate(parts, axis=1))
        # routing tensors packed to [p, n, k] (token tile n, partition p)
        tkp_c = np.ascontiguousarray(
            tkp[sl].reshape(NT, P, 2).transpose(1, 0, 2)).reshape(P, NT * 2)
        tki4 = np.zeros((TC, 4), dtype=np.int32)
        tki4[:, 0] = tki_in[sl, 0]
        tki4[:, 2] = tki_in[sl, 1]
        tki_c = np.ascontiguousarray(
            tki4.reshape(NT, P, 4).transpose(1, 0, 2)).reshape(P, NT * 4)
        in_maps.append(dict(
            xt=xt_c, w1=w1p, w2=w2p, wd=wdp, wu=wup_b, b1=b1p, b2=b2p,
            tkp=tkp_c, tki=tki_c, ident=ident))
    return in_maps


def _ensure_ntff_hook():
    """Register the axon NTFF profile hook if the image's antenv lacks it."""
    import sys
    import types
    try:
        from antenv.axon_hooks import get_axon_ntff_profile_hook  # noqa: F401
        return True
    except ImportError:
        pass
    try:
        from trn_agent_boot.trn_boot import _ntff_profile_via_ctypes
        mod = types.ModuleType("antenv.axon_hooks")
        _hook = [None]
        mod.set_axon_ntff_profile_hook = lambda h: _hook.__setitem__(0, h)
        mod.get_axon_ntff_profile_hook = lambda: _hook[0]
        sys.modules["antenv.axon_hooks"] = mod
        import antenv
        antenv.axon_hooks = mod
        mod.set_axon_ntff_profile_hook(
            _ntff_profile_via_ctypes("/opt/axon/libaxon_pjrt.so"))
        return True
    except Exception:
        return False


def kernel(**inputs):
    from concourse.bass_utils import run_bass_kernel_spmd

    nc = _get_nc()
    in_maps = _make_in_maps(inputs)
    trace = bool(int(os.environ.get("KERNEL_TRACE", "0")))
    if trace and not _ensure_ntff_hook():
        trace = False
    res = run_bass_kernel_spmd(nc, in_maps, list(range(N_CORES)), trace=trace)
    if trace:
        _CACHE["last_result"] = res
    out = np.concatenate([res.results[i]["out"] for i in range(N_CORES)], axis=0)
    return out.reshape(np.asarray(inputs["x"]).shape).astype(np.float32)
